# revision 2
# baseline (speedup 1.0000x reference)
"""DiffGLCM Trainium2 kernel (v2: shared-u row-parity scheme).

Reference: t_j = A_j - A_{j+1} per pixel with A = [1, sigma_1..sigma_63, 0],
GLCM = sum_p t_c(p) outer t_p(p), normalized per image.

Kernel computes S = sum_p A_c(p) outer A_p(p) (64x64 raw sigmoid
co-occurrence; row/col 64 of the 65-wide A are exactly 0 and dropped) on
the PE; the difference is linear, so on host
G[i,j] = S[i,j] - S[i+1,j] - S[i,j+1] + S[i+1,j+1] (S zero-padded).

Core trick vs v1: the sigmoid tensor u is computed ONCE per pixel and
shared between the center and periph roles of the matmul. The (1,1)
offset pairing is made partition-aligned by splitting rows by parity:
  E tile: partition p = image row 2p   (rows 0..254)
  O tile: partition p = image row 2p+1 (rows 1..255)
Even-center pairs (2p,c)x(2p+1,c+1) = E x O at equal partitions (K=128).
Odd-center pairs (2p+1,c)x(2p+2,c+1) need E shifted one partition; that
shifted copy (ES) is produced by a cheap SBUF->SBUF DMA of u_E[1:128]
(matmul operands must start at partition 0, so an offset AP is illegal).

Per tile: DVE+Pool split the bias subtraction (x - j/64) in fp32->bf16,
one fat in-place ACT sigmoid (scale=640) produces u, bin 0 is a const
ones slab (A_0 == 1 exactly). 4 PSUM accumulators (2 matmul families x 2
column parities) keep fp32 accumulation error small; host sums in fp64.
Batch of 16 images -> 2 per NeuronCore x 8 cores.
"""

import sys

sys.path.insert(0, "/opt/trn_rl_repo")

import numpy as np

import concourse.bass as bass
import concourse.mybir as mybir
import concourse.tile as tile
from concourse.bass_utils import run_bass_kernel_spmd

F32 = mybir.dt.float32
BF16 = mybir.dt.bfloat16
H = W = 256
NIMG = 2          # images per core
NB = 64           # matmul width: bin 0 = const ones, bins 1..63 = sigmoids
NSIG = 63         # sigmoid bins
DVE_BINS = 40     # bins 1..DVE_BINS subtracted on vector, rest on gpsimd
N_ACC = 4         # PSUM accumulators per image
HALF = 128        # rows per parity tile


def _build_program(split=True, loop_reps=0):
    import contextlib

    nc = bass.Bass()
    # host passes x as [img, 128, 2, 256]: xs[i, h, p, w] = x[i, 2h+p, w]
    xs = nc.declare_dram_parameter("xs", [NIMG, HALF, 2, W], F32, isOutput=False)
    shift = nc.declare_dram_parameter("shift", [128, NSIG], F32, isOutput=False)
    out = nc.declare_dram_parameter("glcm", [NIMG, N_ACC, NB, NB], F32, isOutput=True)

    with tile.TileContext(nc) as tc:
        with (
            tc.tile_pool(name="const", bufs=1) as const_pool,
            tc.tile_pool(name="x", bufs=2) as x_pool,
            tc.tile_pool(name="u", bufs=2) as u_pool,
            tc.tile_pool(name="ues", bufs=1) as ues_pool,
            tc.tile_pool(name="oub", bufs=2) as out_pool,
            tc.tile_pool(name="ps", bufs=2, space="PSUM") as psum_pool,
        ):
            # shift absorbed into SBUF via one copy so downstream consumers
            # never wait on the DMA queues.
            sh_raw = const_pool.tile([128, NSIG], F32)
            nc.sync.dma_start(sh_raw[:], shift[:])
            sh = const_pool.tile([128, NSIG], F32)
            nc.vector.tensor_copy(sh[:], sh_raw[:])

            rep_ctx = (
                tc.For_i(0, loop_reps, 1) if loop_reps else contextlib.nullcontext()
            )
            with rep_ctx:
              for img in range(NIMG):
                us = {}
                for pi, nm in enumerate(("E", "O")):
                    xt = x_pool.tile([128, W], F32, tag="x" + nm, name="x" + nm)
                    nc.sync.dma_start(xt[:], xs[img, :, pi, :])
                    u = u_pool.tile([128, NB, W], BF16, tag="u" + nm, name="u" + nm)
                    nc.gpsimd.memset(u[:, 0, :], 1.0)
                    xb = xt[:].unsqueeze(1)
                    nc.vector.tensor_sub(
                        u[:, 1 : 1 + DVE_BINS, :],
                        xb.broadcast_to([128, DVE_BINS, W]),
                        sh[:, 0:DVE_BINS]
                        .unsqueeze(2)
                        .broadcast_to([128, DVE_BINS, W]),
                    )
                    nc.gpsimd.tensor_sub(
                        u[:, 1 + DVE_BINS : NB, :],
                        xb.broadcast_to([128, NSIG - DVE_BINS, W]),
                        sh[:, DVE_BINS:NSIG]
                        .unsqueeze(2)
                        .broadcast_to([128, NSIG - DVE_BINS, W]),
                    )
                    # sigma = sigmoid(640*(x - j/64)), in place; bins 0 stays 1
                    nc.scalar.activation(
                        u[:, 1:NB, :],
                        u[:, 1:NB, :],
                        mybir.ActivationFunctionType.Sigmoid,
                        scale=640.0,
                    )
                    us[nm] = u
                # ES: u_E shifted down one partition (row 2p+2), via DMA since
                # matmul operands must start at partition 0. Split across
                # queues by column slices.
                ues = ues_pool.tile([128, NB, W], BF16, tag="ues", name="ues")
                CSL = 64
                for s0 in range(0, W, CSL):
                    nc.sync.dma_start(
                        ues[0:127, :, s0 : s0 + CSL],
                        us["E"][1:128, :, s0 : s0 + CSL],
                    )
                psums = []
                for g in range(N_ACC):
                    pst = psum_pool.tile([NB, NB], F32, tag=f"ps{g}", name=f"ps{g}")
                    psums.append(pst)
                nmm = [0] * N_ACC
                tot = [128, 127, 128, 127]  # ceil/floor of 255 by col parity
                for c in range(W - 1):
                    g = c % 2
                    nc.tensor.matmul(
                        psums[g][:, :],
                        us["E"][:, :, c],
                        us["O"][:, :, c + 1],
                        start=(nmm[g] == 0),
                        stop=(nmm[g] == tot[g] - 1),
                    )
                    nmm[g] += 1
                    g2 = 2 + c % 2
                    nc.tensor.matmul(
                        psums[g2][:, :],
                        us["O"][0:127, :, c],
                        ues[0:127, :, c + 1],
                        start=(nmm[g2] == 0),
                        stop=(nmm[g2] == tot[g2] - 1),
                    )
                    nmm[g2] += 1
                # ob: [64 partitions, N_ACC, 64] - each psum copied to one slot
                ob = out_pool.tile([NB, N_ACC, NB], F32, name="ob")
                for g in range(N_ACC):
                    nc.vector.tensor_copy(ob[:, g, :], psums[g][:, :])
                nc.sync.dma_start(out[img].rearrange("a r c -> r a c"), ob[:])
    if split:
        _split_waits(nc)
    return nc


def _split_waits(nc):
    """This walrus build rejects >1 sync wait on ANY instruction struct
    (even Tile's own end-of-kernel drain). Rewrite every multi-wait
    instruction into a chain of single-wait same-engine drains followed
    by the instruction carrying its last wait.
    """
    n = 0
    for bb in nc.m.functions[0].blocks:
        out = []
        for ins in bb.instructions:
            si = ins.sync_info
            if si is not None and si.on_wait and len(si.on_wait) > 1:
                waits = list(si.on_wait)
                for w in waits[:-1]:
                    out.append(
                        mybir.InstDrain(
                            name=f"waitsplit-{n}",
                            engine=ins.engine,
                            sync_info=mybir.SyncInfo(on_wait=[w], on_update=[]),
                        )
                    )
                    n += 1
                ins.sync_info = mybir.SyncInfo(
                    on_wait=waits[-1:], on_update=list(si.on_update or [])
                )
            out.append(ins)
        bb.instructions[:] = out
    return n


def make_in_maps(x):
    # shift[p, j-1] = j/64 for j = 1..63, replicated over 128 partitions
    sv = (np.arange(1, NB, dtype=np.float32) / np.float32(NB))[None, :]
    shift = np.ascontiguousarray(np.broadcast_to(sv, (128, NSIG)))
    xr = x.reshape(16, HALF, 2, W)  # row 2h+p -> [h, p]
    return [
        {"xs": np.ascontiguousarray(xr[2 * k : 2 * k + 2]), "shift": shift}
        for k in range(8)
    ]


def _finish_host(raw):
    # raw: [16, N_ACC, 64, 64] -- fp64-sum accumulators, zero-pad to 65x65,
    # 2D second difference, then normalize.
    s = raw.astype(np.float64).sum(axis=1)  # [16, 64, 64]
    sp = np.zeros((s.shape[0], NB + 1, NB + 1), dtype=np.float64)
    sp[:, :NB, :NB] = s
    g = sp[:, :NB, :NB] - sp[:, 1:, :NB] - sp[:, :NB, 1:] + sp[:, 1:, 1:]
    g = g / g.sum(axis=(1, 2), keepdims=True)
    return g.astype(np.float32)


_NC = None


def kernel(x, offset_r=1, offset_c=1, **_):
    global _NC
    assert int(offset_r) == 1 and int(offset_c) == 1
    x = np.ascontiguousarray(np.asarray(x, dtype=np.float32).reshape(16, H, W))
    if _NC is None:
        _NC = _build_program()
    res = run_bass_kernel_spmd(_NC, make_in_maps(x), core_ids=list(range(8)))
    raw = np.concatenate([r["glcm"] for r in res.results], axis=0)
    return _finish_host(raw).reshape(16, 1, NB, NB, 1)


if __name__ == "__main__":
    _build_program()
    print("build OK")
